# revision 5
# baseline (speedup 1.0000x reference)
"""Trainium2 Bass kernel for a causal multi-head attention block.

Same contract as kernel.py. Key change vs baseline: the attention phase's
per-128-key-tile pipeline is merged into GM-tile groups ([128, GM*512]
score PSUM tiles, one Exp activation per group) to amortize the ACT
engine's ~350-cycle per-instruction overhead — sim showed phase 2 is
ACT(exp)-bound. Diagonal groups compute full-width scores (finite values)
and are masked post-exp by a DVE mul with a static staircase mask, so no
uninitialized PSUM is ever read. Rowsum accumulation runs on [128, GM*512]
rings (fewer, wider DVE ops); the finalize rowsum matmul moves to the accp
PSUM pool to free PSUM banks for the wider score tiles.
"""

import sys

sys.path.insert(0, "/opt/trn_rl_repo")

import numpy as np
import ml_dtypes

import concourse.bass as bass
from concourse import bacc
import concourse.mybir as mybir
import concourse.tile as tile
from concourse.bass_utils import run_bass_kernel_spmd
from concourse.masks import make_identity

N = 4096          # tokens
C = 2048          # model dim
H = 16            # heads
D = 128           # head dim
NCORES = 8
HPC = H // NCORES  # heads per core = 2
NB = N // 512      # 8 n-blocks (query blocks of 512)
NT = N // 128      # 32 m-tiles (key tiles of 128)
CT = C // 128      # 16 contraction tiles for the qkv projection
SCALE = float(D) ** -0.5
SIN_TIME = 10000.0

BF16 = mybir.dt.bfloat16
F32 = mybir.dt.float32
FP16 = mybir.dt.float16

_CACHE = {}


def _cache_tag(cfg):
    import zlib
    with open(__file__, "rb") as f:
        h = zlib.crc32(f.read())
    h = zlib.crc32(repr(sorted(cfg.items())).encode(), h)
    return 16 + (h % 4096)

CFG = dict(
    rope_swap="act",      # "act": partition-offset ACT copies; "dma": sbuf dma
    y_dtype="bf16",       # partial-output dtype ("bf16" | "f32")
    gm=2,                 # key tiles merged per scores/exp group (1|2|4)
    pipe_depth=2,         # groups in flight between scores-MM and AV-MM
                          # (MUST be <= stp: deeper lookahead head-of-line
                          # blocks the in-order PE queue on HW)
    interleave=True,      # spread proj matmuls through the next block's groups
    attn_per_unit=0,      # attention groups consumed per phase-1 unit
    mask_mode="narrow",   # "narrow": per-sub-tile qoff narrowing + 128x128 tri
                          # mask; "asel" Pool affine_select; "mul" DVE wide mul
    r_pool_frac=0.0,      # fraction of R-accum ops routed to the Pool engine
    prime=True,           # emit first score groups during the last ph1 block
    exp_narrow=True,      # narrowed per-sub-tile exp on diagonal groups
    h_interleave=False,   # alternate heads in the group stream (2 live chains)
    fdelay=3,             # group-consumes between block's last group and finalize
    accp=4,               # PSUM banks for accumulators (accp + gm*stp <= 8)
    stp=2,                # score-group PSUM buffers (each gm banks)
    y_half_dma=True,      # y DMA per 1024-col half (else per row-tile)
    v_path="transpose",   # "direct": token-major v MMs; "transpose": PE transp
    ys_engine="any",      # engine for proj-output copies ("any" | "dve")
    fscale="dve",         # engine for the 1/rowsum output scale ("dve"|"gpsimd")
    etp=8,                # exp-output ring depth (SBUF, gm KB/partition each)
    probe="full",         # "noattn": timing probe, skips attention phase
    repeat=1,             # wrap whole body in a hardware loop (timing calib)
    io_stub=False,        # timing-only: big I/O tensors become Internal
)


def build_nc(**overrides):
    cfg = dict(CFG)
    cfg.update(overrides)
    ydt = BF16 if cfg["y_dtype"] == "bf16" else F32
    GM = cfg["gm"]
    assert 4 % GM == 0

    nc = bacc.Bacc(None, target_bir_lowering=False)

    in_kind = "Internal" if cfg["io_stub"] else "ExternalInput"
    out_kind = "Internal" if cfg["io_stub"] else "ExternalOutput"
    xT_d = nc.dram_tensor("xT", [C, N], FP16, kind=in_kind)
    wqkvT_d = nc.dram_tensor("wqkvT", [C, 6 * D], FP16, kind=in_kind)
    wpT_d = nc.dram_tensor("wpT", [HPC * D, C], FP16, kind=in_kind)
    cosT_d = nc.dram_tensor("cosT", [D, N], FP16, kind=in_kind)
    sinT_d = nc.dram_tensor("sinT", [D, N], FP16, kind=in_kind)
    y_d = nc.dram_tensor("y", [N, C], ydt, kind=out_kind)
    t0_d = None
    if cfg["io_stub"]:
        t0_d = nc.dram_tensor("t0", [1, 1], FP16, kind="ExternalOutput")
    # cache-buster: the neuron compile cache hashes only tensor shapes
    nc.dram_tensor("cachetag", [_cache_tag(cfg), 1], F32, kind="ExternalInput")

    with tile.TileContext(nc) as tc:
        with (
            tc.tile_pool(name="persist", bufs=1) as persist,
            tc.tile_pool(name="xtp", bufs=2) as xtp,
            tc.tile_pool(name="etp", bufs=cfg["etp"]) as etp,
            tc.tile_pool(name="ropep", bufs=3) as ropep,
            tc.tile_pool(name="misc", bufs=2) as misc,
            tc.tile_pool(name="ysp", bufs=4) as ysp,
            tc.tile_pool(name="accp", bufs=cfg["accp"], space="PSUM") as accp,
            tc.tile_pool(name="stp", bufs=cfg["stp"], space="PSUM") as stp,
        ):
            import contextlib

            loop_ctx = (
                tc.For_i(0, cfg["repeat"], 1,
                         hint_engines=tuple(nc.engines.keys()))
                if cfg["repeat"] > 1 else contextlib.nullcontext()
            )
            with loop_ctx:
                # ---- first x block early so the PE can start immediately ----
                def load_x_chunks(t, j, lo, hi, step):
                    for s in range(lo, hi):
                        nc.sync.dma_start(
                            t[:, s * step:(s + 1) * step, :],
                            xT_d[s * step * 128:(s + 1) * step * 128,
                                 j * 512:(j + 1) * 512].rearrange(
                                "(t p) n -> p t n", p=128
                            ),
                        )

                def load_x_block(j, nchunks=1):
                    t = xtp.tile([128, CT, 512], FP16, tag="xt", name=f"xt_{j}")
                    load_x_chunks(t, j, 0, nchunks, CT // nchunks)
                    return t

                wq_s = []
                for u in range(4):
                    w = persist.tile([128, CT, 128], FP16, tag=f"wq{u}", name=f"wq{u}")
                    wq_s.append(w)
                wv = persist.tile([128, CT, 2 * D], FP16, tag="wv", name="wv")

                def load_wq(u, lo=0, hi=4, step=4):
                    for s in range(lo, hi):
                        nc.sync.dma_start(
                            wq_s[u][:, s * step:(s + 1) * step, :],
                            wqkvT_d[s * step * 128:(s + 1) * step * 128,
                                    u * D:(u + 1) * D].rearrange(
                                "(t p) d -> p t d", p=128),
                        )

                cosT = persist.tile([128, N], FP16, tag="cosT", name="cosT")
                sinT = persist.tile([128, N], FP16, tag="sinT", name="sinT")
                xt3 = xtp.tile([128, CT, 512], FP16, tag="xt", name="xt_0")
                # finest-grain first loads so unit-0 ct-0 matmuls start asap
                load_wq(0, 0, 1)
                load_x_chunks(xt3, 0, 0, 1, 2)
                load_wq(0, 1, 2)
                load_x_chunks(xt3, 0, 1, 2, 2)
                load_wq(0, 2, 4)
                load_x_chunks(xt3, 0, 2, 4, 2)
                load_wq(1, 0, 2)
                load_x_chunks(xt3, 0, 4, 6, 2)
                load_wq(1, 2, 4)
                load_x_chunks(xt3, 0, 6, 8, 2)
                load_wq(2)
                load_wq(3)
                for s in range(2):
                    nc.sync.dma_start(
                        wv[:, s * 8:(s + 1) * 8, :],
                        wqkvT_d[s * 8 * 128:(s + 1) * 8 * 128,
                                4 * D:6 * D].rearrange("(t p) d -> p t d", p=128),
                    )
                nc.sync.dma_start(cosT[:, 0:512], cosT_d[:, 0:512])
                nc.sync.dma_start(sinT[:, 0:512], sinT_d[:, 0:512])
                ones = persist.tile([128, 1], FP16, tag="ones", name="ones")
                nc.vector.memset(ones[:], 1.0)
                identity = None
                if cfg["v_path"] == "transpose":
                    identity = persist.tile([128, 128], FP16, tag="identity",
                                            name="identity")
                    make_identity(nc, identity[:])
                # staircase masks for the diagonal groups: sub-tile i of
                # diagonal group s keeps q >= (s*GM + i)*128 + p
                masks = []
                if cfg["mask_mode"] == "mul":
                    for s in range(4 // GM):
                        m = persist.tile([128, GM, 512], FP16, tag=f"mask{s}",
                                         name=f"mask{s}")
                        nc.gpsimd.memset(m[:], 1.0)
                        pattern = ([[-128, GM], [1, 512]] if GM > 1
                                   else [[1, 512]])
                        nc.gpsimd.affine_select(
                            out=m[:], in_=m[:],
                            pattern=pattern,
                            compare_op=mybir.AluOpType.is_ge,
                            fill=0.0,
                            base=-128 * GM * s,
                            channel_multiplier=-1,
                        )
                        masks.append(m)
                mask_tri = None
                if cfg["mask_mode"] == "narrow":
                    # single lower-triangle 0/1 mask for diagonal sub-tiles
                    mask_tri = persist.tile([128, 128], FP16, tag="mask",
                                            name="mask_tri")
                    nc.gpsimd.memset(mask_tri[:], 1.0)
                    nc.gpsimd.affine_select(
                        out=mask_tri[:], in_=mask_tri[:],
                        pattern=[[1, 128]],
                        compare_op=mybir.AluOpType.is_ge,
                        fill=0.0,
                        base=0,
                        channel_multiplier=-1,
                    )

                # persistent activations: q_h0, q_h1, k_h0, k_h1
                qk_store = []
                for u in range(4):
                    t = persist.tile([128, N], FP16, tag=f"qk{u}", name=f"qk{u}")
                    qk_store.append(t)
                v_store = persist.tile([128, NT, 2 * D], FP16, tag="v", name="v")
                ots = []
                for h in range(HPC):
                    t = persist.tile([128, N], FP16, tag=f"ot{h}", name=f"ot{h}")
                    ots.append(t)
                wp_s = []
                for h in range(HPC):
                    w = persist.tile([128, C], FP16, tag=f"wp{h}", name=f"wp{h}")
                    wp_s.append(w)

                # ---- phase-1 building blocks (qkv projection + rope + v) ----
                def ph1_prefetch(j):
                    if j + 1 < NB:
                        nc.sync.dma_start(
                            cosT[:, (j + 1) * 512:(j + 2) * 512],
                            cosT_d[:, (j + 1) * 512:(j + 2) * 512])
                        nc.sync.dma_start(
                            sinT[:, (j + 1) * 512:(j + 2) * 512],
                            sinT_d[:, (j + 1) * 512:(j + 2) * 512])
                        return load_x_block(j + 1)
                    return None

                def ph1_qk_unit(j, u, xt):
                    ps = accp.tile([128, 512], F32, tag="acc", name=f"qkvps_{j}_{u}")
                    for ct in range(CT):
                        nc.tensor.matmul(
                            ps[:], wq_s[u][:, ct, :], xt[:, ct, :],
                            start=(ct == 0), stop=(ct == CT - 1),
                        )
                    qswap = ropep.tile([128, 512], F32, tag="qswap", name=f"qswap_{j}_{u}")
                    if cfg["rope_swap"] == "act":
                        nc.scalar.copy(qswap[0:64, :], ps[64:128, :])
                        nc.scalar.copy(qswap[64:128, :], ps[0:64, :])
                    else:
                        qraw = ropep.tile([128, 512], F32, tag="qraw", name=f"qraw_{j}_{u}")
                        nc.scalar.copy(qraw[:], ps[:])
                        nc.sync.dma_start(qswap[0:64, :], qraw[64:128, :])
                        nc.sync.dma_start(qswap[64:128, :], qraw[0:64, :])
                    dst = qk_store[u][:, j * 512:(j + 1) * 512]
                    nc.vector.tensor_mul(dst, ps[:], cosT[:, j * 512:(j + 1) * 512])
                    # fp16 ut: the dst += ut add becomes all-SBUF 2-byte (2x)
                    ut = ropep.tile([128, 512], FP16, tag="ut", name=f"ut_{j}_{u}")
                    nc.gpsimd.tensor_mul(ut[:], qswap[:], sinT[:, j * 512:(j + 1) * 512])
                    nc.vector.tensor_add(dst, dst, ut[:])

                def ph1_v_head(j, h, xt):
                    ps = accp.tile([128, 512], F32, tag="acc", name=f"vps_{j}_{h}")
                    for ct in range(CT):
                        nc.tensor.matmul(
                            ps[:], wv[:, ct, h * 128:(h + 1) * 128],
                            xt[:, ct, :],
                            start=(ct == 0), stop=(ct == CT - 1),
                        )
                    vtmp = misc.tile([128, 512], FP16, tag="vtmp", name=f"vtmp_{j}_{h}")
                    nc.scalar.copy(vtmp[:], ps[:])
                    for s in range(4):
                        pst = stp.tile([128, GM, 512], FP16, tag="st",
                                       name=f"vt_{j}_{h}_{s}")
                        nc.tensor.transpose(
                            pst[:, 0, 0:128], vtmp[:, s * 128:(s + 1) * 128],
                            identity[:],
                        )
                        nc.vector.tensor_copy(
                            out=v_store[:, j * 4 + s, h * 128:(h + 1) * 128],
                            in_=pst[:, 0, 0:128],
                        )

                def ph1_v_nt(j, nt, xt):
                    psv = accp.tile([128, 2 * D], F32, tag="acc", name=f"vps_{j}_{nt}")
                    for ct in range(CT):
                        nc.tensor.matmul(
                            psv[:], xt[:, ct, nt * 128:(nt + 1) * 128],
                            wv[:, ct, :],
                            start=(ct == 0), stop=(ct == CT - 1),
                        )
                    nc.vector.tensor_copy(
                        out=v_store[:, j * 4 + nt, :], in_=psv[:])

                # ---- attention pipeline at group (GM key tiles) granularity.
                from collections import deque

                state = {}

                def get_state(j, h):
                    if (j, h) not in state:
                        state[(j, h)] = dict(
                            ot=accp.tile([128, 512], F32, tag="acc",
                                         name=f"ot_{h}_{j}"),
                            Rts=[misc.tile([128, GM, 512], FP16, tag=f"R{ri}",
                                           name=f"R{ri}_{h}_{j}", bufs=2)
                                 for ri in range(2)],
                            Rinit=[False, False],
                            Rq=[[None] * GM, [None] * GM],
                        )
                    return state[(j, h)]

                def emit_scores(j, h, g):
                    d0 = (4 * j) // GM  # first diagonal group index
                    mode = cfg["mask_mode"]
                    stg = stp.tile([128, GM, 512], F32, tag="st",
                                   name=f"st_{h}_{j}_{g}")
                    for i in range(GM):
                        t = g * GM + i
                        qoff = (max(0, (t - 4 * j) * 128)
                                if mode != "mul" else 0)
                        nc.tensor.matmul(
                            stg[:, i, qoff:],
                            qk_store[2 + h][:, t * 128:(t + 1) * 128],
                            qk_store[h][:, j * 512 + qoff:(j + 1) * 512],
                            start=True, stop=True,
                        )
                    et = etp.tile([128, GM, 512], FP16, tag="et",
                                  name=f"et_{h}_{j}_{g}")
                    if mode == "narrow" and g >= d0 and cfg["exp_narrow"]:
                        # per-sub-tile narrowed exp: skips the never-consumed
                        # garbage columns of diagonal groups
                        for i in range(GM):
                            t = g * GM + i
                            qoff = max(0, (t - 4 * j) * 128)
                            nc.scalar.activation(
                                et[:, i, qoff:], stg[:, i, qoff:],
                                mybir.ActivationFunctionType.Exp, scale=SCALE,
                            )
                    else:
                        nc.scalar.activation(
                            et[:], stg[:],
                            mybir.ActivationFunctionType.Exp, scale=SCALE,
                        )
                    if g >= d0:
                        s = g - d0
                        if mode == "asel":
                            # zero q < (s*GM+i)*128 + p; also overwrites the
                            # exp-of-uninitialized-PSUM region with 0
                            pattern = ([[-128, GM], [1, 512]] if GM > 1
                                       else [[1, 512]])
                            nc.gpsimd.affine_select(
                                out=et[:], in_=et[:],
                                pattern=pattern,
                                compare_op=mybir.AluOpType.is_ge,
                                fill=0.0,
                                base=-128 * GM * s,
                                channel_multiplier=-1,
                            )
                        elif mode == "mul":
                            nc.vector.tensor_mul(et[:], et[:], masks[s][:])
                        else:  # narrow: tri-mask only the diagonal 128-block
                            for i in range(GM):
                                qoff = (g * GM + i - 4 * j) * 128
                                nc.vector.tensor_mul(
                                    et[:, i, qoff:qoff + 128],
                                    et[:, i, qoff:qoff + 128], mask_tri[:],
                                )
                    return (j, h, g, et)

                rpool = dict(n=0.0)

                def r_engine():
                    rpool["n"] += cfg["r_pool_frac"]
                    if rpool["n"] >= 1.0:
                        rpool["n"] -= 1.0
                        return nc.gpsimd
                    return nc.vector

                def emit_consume(j, h, g, et):
                    s = get_state(j, h)
                    ri = g % 2
                    R = s["Rts"][ri]
                    if cfg["mask_mode"] == "narrow":
                        Rq = s["Rq"][ri]
                        if all(q == 0 for q in Rq) and g < (4 * j) // GM:
                            # ring fully live, full-width group: one wide add
                            r_engine().tensor_add(R[:], R[:], et[:])
                        else:
                            for i in range(GM):
                                t = g * GM + i
                                qoff = max(0, (t - 4 * j) * 128)
                                if Rq[i] is None:
                                    Rq[i] = qoff
                                    nc.vector.tensor_copy(
                                        out=R[:, i, qoff:], in_=et[:, i, qoff:])
                                else:
                                    nc.vector.tensor_add(
                                        R[:, i, qoff:], R[:, i, qoff:],
                                        et[:, i, qoff:])
                    elif not s["Rinit"][ri]:
                        s["Rinit"][ri] = True
                        nc.vector.tensor_copy(out=R[:], in_=et[:])
                    else:
                        r_engine().tensor_add(R[:], R[:], et[:])
                    for i in range(GM):
                        t = g * GM + i
                        qoff = max(0, (t - 4 * j) * 128)
                        nc.tensor.matmul(
                            s["ot"][:, qoff:],
                            v_store[:, t, h * 128:(h + 1) * 128],
                            et[:, i, qoff:],
                            start=(t == 0), stop=(t == 4 * j + 3),
                            skip_group_check=True,
                        )

                def finalize(j, h):
                    s = state.pop((j, h))
                    Rts, Rinit = s["Rts"], s["Rinit"]
                    R = Rts[0]
                    if cfg["mask_mode"] == "narrow":
                        Rq0, Rq1 = s["Rq"]
                        # ring0 always holds group 0 (sub-tile i from qoff
                        # i*128 at j=0, else 0); ring1 covers >= ring0
                        if any(q is not None for q in Rq1):
                            if all(q == 0 for q in Rq0) and all(
                                    q == 0 for q in Rq1):
                                nc.vector.tensor_add(R[:], R[:], Rts[1][:])
                            else:
                                for i in range(GM):
                                    q1 = Rq1[i]
                                    if q1 is None:
                                        continue
                                    nc.vector.tensor_add(
                                        R[:, i, q1:], R[:, i, q1:],
                                        Rts[1][:, i, q1:])
                        for i in range(1, GM):
                            qi = Rq0[i]
                            nc.vector.tensor_add(
                                R[:, 0, qi:], R[:, 0, qi:], R[:, i, qi:])
                    else:
                        if Rinit[1]:
                            nc.vector.tensor_add(R[:], R[:], Rts[1][:])
                        # fold GM sub-tiles down to [128, 512]
                        if GM == 4:
                            nc.vector.tensor_add(
                                R[:, 0:2, :], R[:, 0:2, :], R[:, 2:4, :])
                        if GM >= 2:
                            nc.vector.tensor_add(
                                R[:, 0, :], R[:, 0, :], R[:, 1, :])
                    rs_ps = accp.tile([128, 512], F32, tag="acc",
                                      name=f"rs_{h}_{j}")
                    nc.tensor.matmul(
                        rs_ps[0:1, :], ones[:], R[:, 0, :],
                        start=True, stop=True, skip_group_check=True,
                    )
                    recip = misc.tile([1, 512], F32, tag="recip",
                                      name=f"recip_{h}_{j}")
                    nc.vector.reciprocal(recip[:], rs_ps[0:1, :])
                    rb = misc.tile([128, 512], F32, tag="rb", name=f"rb_{h}_{j}")
                    nc.gpsimd.partition_broadcast(rb[:], recip[:], channels=128)
                    if j == NB - 1 and h == HPC - 1:
                        # tail: interleave this head's per-row-tile scale with
                        # the block's projection chunks so the proj matmuls
                        # don't wait for the whole scale chain
                        gen = proj_gen(j)
                        for sq in range(4):
                            c0, c1 = sq * 128, (sq + 1) * 128
                            nc.vector.tensor_mul(
                                ots[h][:, j * 512 + c0:j * 512 + c1],
                                s["ot"][:, c0:c1], rb[:, c0:c1],
                            )
                            for _ in range(4):
                                next(gen, None)
                        for _ in gen:
                            pass
                        return False  # proj already emitted
                    if j == NB - 1:
                        # split per row-tile so the tail projection can start
                        # as soon as its own row-tile is scaled
                        for sq in range(4):
                            c0, c1 = sq * 128, (sq + 1) * 128
                            nc.vector.tensor_mul(
                                ots[h][:, j * 512 + c0:j * 512 + c1],
                                s["ot"][:, c0:c1], rb[:, c0:c1],
                            )
                    else:
                        feng = (nc.gpsimd if cfg["fscale"] == "gpsimd"
                                else nc.vector)
                        feng.tensor_mul(
                            ots[h][:, j * 512:(j + 1) * 512], s["ot"][:], rb[:]
                        )
                    return True

                def proj_gen(j):
                    for nt in range(4 * j, 4 * j + 4):
                        ys = ysp.tile([128, C], ydt, tag="ys", name=f"ys_{nt}")
                        for cc in range(4):
                            py = accp.tile([128, 512], F32, tag="acc",
                                           name=f"py_{nt}_{cc}")
                            for h in range(HPC):
                                nc.tensor.matmul(
                                    py[:], ots[h][:, nt * 128:(nt + 1) * 128],
                                    wp_s[h][:, cc * 512:(cc + 1) * 512],
                                    start=(h == 0), stop=(h == HPC - 1),
                                    skip_group_check=True,
                                )
                            ys_eng = (nc.vector if cfg["ys_engine"] == "dve"
                                      else nc.any)
                            ys_eng.tensor_copy(
                                out=ys[:, cc * 512:(cc + 1) * 512], in_=py[:]
                            )
                            if cfg["y_half_dma"] and cc % 2:
                                nc.sync.dma_start(
                                    y_d[nt * 128:(nt + 1) * 128,
                                        (cc - 1) * 512:(cc + 1) * 512],
                                    ys[:, (cc - 1) * 512:(cc + 1) * 512])
                            yield
                        if not cfg["y_half_dma"]:
                            nc.sync.dma_start(
                                y_d[nt * 128:(nt + 1) * 128, :], ys[:])

                L = cfg["pipe_depth"]
                proj_q = deque()

                def step_proj():
                    while proj_q:
                        ent = proj_q[0]
                        ent[2] += 1
                        if ent[2] % ent[1]:
                            return
                        try:
                            next(ent[0])
                            return
                        except StopIteration:
                            proj_q.popleft()

                def run_finalize(jj, hh):
                    if not finalize(jj, hh):
                        return  # tail path emitted its own projection
                    if hh == HPC - 1:
                        if cfg["interleave"]:
                            # next block has ~2*(4(jj+1)+4)/GM consumes for
                            # 16 proj chunks
                            ncons = 2 * (4 * (jj + 1) + 4) // GM
                            stride = max(1, ncons // 17)
                            proj_q.append([proj_gen(jj), stride, 0])
                        else:
                            for _ in proj_gen(jj):
                                pass

                FDELAY = cfg["fdelay"]
                pending = deque()
                fin_q = deque()
                backlog = deque()
                S = dict(g=0)

                def attn_step():
                    while backlog and len(pending) <= L:
                        pending.append(emit_scores(*backlog.popleft()))
                    if not pending:
                        return False
                    j, h, g, et = pending.popleft()
                    emit_consume(j, h, g, et)
                    S["g"] += 1
                    if g == (4 * j + 4) // GM - 1:
                        fin_q.append((j, h, S["g"]))
                    if fin_q and S["g"] - fin_q[0][2] >= FDELAY:
                        jj, hh, _ = fin_q.popleft()
                        run_finalize(jj, hh)
                    if cfg["interleave"]:
                        step_proj()
                    return True

                APU = cfg["attn_per_unit"]
                xt_next = xt3
                for j in range(NB):
                    xt = xt_next
                    for u in range(4):
                        ph1_qk_unit(j, u, xt)
                        if u == 0:
                            xt_next = ph1_prefetch(j)
                            if j == 1:
                                for h in range(HPC):
                                    nc.sync.dma_start(
                                        wp_s[h][:], wpT_d[h * D:(h + 1) * D, :])
                        for _ in range(APU):
                            if not attn_step():
                                break
                        if cfg["prime"] and j == NB - 1:
                            # fill the scores->exp pipeline with block-0 groups
                            # so AV can start the moment phase 1 drains
                            while backlog and len(pending) <= L:
                                pending.append(emit_scores(*backlog.popleft()))
                    if cfg["v_path"] == "direct":
                        for nt in range(4):
                            ph1_v_nt(j, nt, xt)
                            for _ in range(APU):
                                if not attn_step():
                                    break
                    else:
                        for h in range(HPC):
                            ph1_v_head(j, h, xt)
                            for _ in range(2 * APU):
                                if not attn_step():
                                    break
                    if cfg["probe"] == "full":
                        if cfg["h_interleave"]:
                            # alternate heads: two independent R/ot chains in
                            # flight -> each chain sees 2x the latency slack
                            for g in range((4 * j + 4) // GM):
                                for h in range(HPC):
                                    backlog.append((j, h, g))
                        else:
                            for h in range(HPC):
                                for g in range((4 * j + 4) // GM):
                                    backlog.append((j, h, g))

                if cfg["probe"] == "noattn":
                    for h in range(HPC):
                        nc.vector.memset(ots[h][:], 0.5)
                    for j in range(NB):
                        for _ in proj_gen(j):
                            pass
                while attn_step():
                    pass
                while fin_q:
                    jj, hh, _ = fin_q.popleft()
                    run_finalize(jj, hh)
                while proj_q:
                    try:
                        next(proj_q[0][0])
                    except StopIteration:
                        proj_q.popleft()

            if t0_d is not None:
                nc.sync.dma_start(t0_d[:, :], ones[0:1, 0:1])

    nc.finalize()
    return nc


def _rope_tables():
    i = np.arange(D)
    denom = np.power(SIN_TIME, 2 * (i // 2) / D)
    pe = np.arange(N)[:, None] / denom[None, :]
    sin = np.sin(pe[:, 0::2])
    cos = np.cos(pe[:, 1::2])
    sin_pos = np.repeat(sin, 2, axis=1)  # [N, D]
    cos_pos = np.repeat(cos, 2, axis=1)
    sin_signed = sin_pos.copy()
    sin_signed[:, 0::2] *= -1.0
    perm = np.concatenate([np.arange(0, D, 2), np.arange(1, D, 2)])
    cosT = np.ascontiguousarray(cos_pos.T[perm, :]).astype(np.float16)
    sinT = np.ascontiguousarray(sin_signed.T[perm, :]).astype(np.float16)
    return cosT, sinT, perm


def prep_in_maps(x, W_qkv, W_proj):
    fp = np.float16
    cosT, sinT, perm = _rope_tables()
    xT = np.ascontiguousarray(x.T).astype(fp)
    WpT = W_proj.T  # [C(dd), C(out)]
    in_maps = []
    for c in range(NCORES):
        h0, h1 = HPC * c, HPC * c + 1
        blocks = []
        for sec in (0, 1):  # q, k: deinterleave-permuted rows
            for h in (h0, h1):
                blk = W_qkv[sec * C + h * D: sec * C + (h + 1) * D, :]
                blocks.append(blk[perm, :])
        for h in (h0, h1):  # v: unpermuted
            blocks.append(W_qkv[2 * C + h * D: 2 * C + (h + 1) * D, :])
        shard = np.concatenate(blocks, axis=0)  # [768, C]
        wqkvT = np.ascontiguousarray(shard.T).astype(fp)  # [C, 768]
        wpT = np.ascontiguousarray(
            WpT[h0 * D:(h1 + 1) * D, :]
        ).astype(fp)  # [256, C]
        in_maps.append(
            {"xT": xT, "wqkvT": wqkvT, "wpT": wpT, "cosT": cosT, "sinT": sinT}
        )
    return in_maps


def add_cachetag(in_maps, cfg=None):
    tag = _cache_tag(dict(CFG, **(cfg or {})))
    for m in in_maps:
        m["cachetag"] = np.zeros((tag, 1), np.float32)
    return in_maps


def kernel(x, W_qkv, W_proj, b_proj):
    x = np.asarray(x, dtype=np.float32)
    W_qkv = np.asarray(W_qkv, dtype=np.float32)
    W_proj = np.asarray(W_proj, dtype=np.float32)
    b_proj = np.asarray(b_proj, dtype=np.float32)

    if "nc" not in _CACHE:
        _CACHE["nc"] = build_nc()
    nc = _CACHE["nc"]
    in_maps = add_cachetag(prep_in_maps(x, W_qkv, W_proj))
    res = run_bass_kernel_spmd(nc, in_maps, core_ids=list(range(NCORES)))
    parts = np.stack(
        [res.results[i]["y"].astype(np.float32) for i in range(NCORES)], axis=0
    )
    y = parts.sum(axis=0, dtype=np.float64).astype(np.float32)
    return y + b_proj[None, :]
